# revision 20
# baseline (speedup 1.0000x reference)
"""Trainium2 Bass kernel for the per-cluster Lorentz boost module.

out[b,c,i] = B[c,i,j] @ T[b,c,j], B derived from per-cluster boost
vectors Bo[c].  Boost matrix closed form:
    B = [[G0, -D n^T], [-D n, I + A n n^T]]
    mag = clip(|Bo|, eps, 1-eps), n = Bo/mag, g = 1/sqrt(1-mag^2)
    A = g-1, D = g*mag, G0 = 1 + A*(n.n)

Algebra (keeps every Vector-engine pass contiguous / full rate):
with lam = -D/A and s~ = lam*x0 + n.x:
    out_k = x_k + A*n_k*s~                      (k = 1..3, exact)
    out_0 = (1+eps)*x0 + (-D)*s~,  eps = (G0-1) + D*lam
Per 128-row tile:
    P    = C1 * X            (C1 = [lam, n1, n2, n3] interleaved)
    x0  *= (1+eps)           (in-place, slot-0 columns of X)
    s~   = segmented_reduce4(P)
    Tps  = A4 * bcast(s~)    (A4 = [-D, A n1, A n2, A n3]) -> PSUM chunks
    O    = Tps + X'          (PE identity-matmul accumulate onto PSUM)
    evict PSUM -> SBUF       (Scalar engine), then store DMA.
GpSimd is deliberately idle: its SW loops contend with the Vector
engine's SBUF ports (measured 2-3.6x DVE slowdown).

Sharding: pure data parallel over batch (8192 -> 8 x 1024 rows/core).
"""

import os
import sys

import numpy as np

_TRN_REPO = "/opt/trn_rl_repo"
if _TRN_REPO not in sys.path:
    sys.path.append(_TRN_REPO)

os.environ.setdefault("TRN_TYPE", "TRN2")

EPS = 1e-7

N_CORES = 8
B_FULL = 8192
B_CORE = B_FULL // N_CORES  # 1024 batch rows per core
C = 1024                    # clusters
FD = C * 4                  # free dim of a batch tile
N_TILES = B_CORE // 128     # 8 tiles of [128, 4096] per core
N_CHUNK = FD // 512         # 8 psum chunks per tile


def _coef_from_bo(Bo: np.ndarray) -> np.ndarray:
    """Per-cluster coefficients (float64 math, fp32 results), replicated
    across partitions: [C1 | A4 | 1+eps] -> (128, 2*FD + C)."""
    Bo = np.asarray(Bo, dtype=np.float32).astype(np.float64)
    mag = np.sqrt(np.sum(Bo * Bo, axis=1, keepdims=True))
    mag = np.clip(mag, EPS, 1.0 - EPS)
    n = Bo / mag                                   # (C,3)
    g = 1.0 / np.sqrt(1.0 - mag * mag)             # (C,1)
    A = g - 1.0
    D = g * mag
    nn = np.sum(n * n, axis=1, keepdims=True)
    G0 = 1.0 + A * nn
    lam32 = (-D / A).astype(np.float32)
    # eps cancels the realized -D*lam32 x0 cross-term exactly
    eps = ((G0 - 1.0) + D * lam32.astype(np.float64))

    C1 = np.empty((C, 4), dtype=np.float32)
    C1[:, 0] = lam32[:, 0]
    C1[:, 1:] = n.astype(np.float32)
    A4 = np.empty((C, 4), dtype=np.float32)
    A4[:, 0] = (-D[:, 0]).astype(np.float32)
    A4[:, 1:] = (A * n).astype(np.float32)
    one_eps = (1.0 + eps[:, 0]).astype(np.float32)

    def rep(row):
        return np.ascontiguousarray(np.broadcast_to(row.reshape(1, -1), (128, row.size)))

    return {"c1": rep(C1), "a4": rep(A4), "eps1": rep(one_eps)}


_CACHED = {}


def _build_program():
    if "nc" in _CACHED:
        return _CACHED["nc"]

    import concourse.bacc as bacc
    import concourse.mybir as mybir
    import concourse.tile as tile

    f32 = mybir.dt.float32
    mult = mybir.AluOpType.mult
    add = mybir.AluOpType.add

    nc = bacc.Bacc("TRN2", target_bir_lowering=False, debug=False)

    t_in = nc.dram_tensor("t", [B_CORE, FD], f32, kind="ExternalInput").ap()
    c1_in = nc.dram_tensor("c1", [128, FD], f32, kind="ExternalInput").ap()
    a4_in = nc.dram_tensor("a4", [128, FD], f32, kind="ExternalInput").ap()
    eps_in = nc.dram_tensor("eps1", [128, C], f32, kind="ExternalInput").ap()
    ident_in = nc.dram_tensor("ident", [128, 128], f32, kind="ExternalInput").ap()
    o_out = nc.dram_tensor("o", [B_CORE, FD], f32, kind="ExternalOutput").ap()

    t_tiled = t_in.rearrange("(n p) m -> n p m", p=128)
    o_tiled = o_out.rearrange("(n p) m -> n p m", p=128)

    with tile.TileContext(nc) as tc:
        with (
            tc.tile_pool(name="coefp", bufs=1) as coefp,
            tc.tile_pool(name="xp", bufs=3) as xp,
            tc.tile_pool(name="pp", bufs=3) as pp,
            tc.tile_pool(name="op", bufs=3) as op,
            tc.tile_pool(name="sp", bufs=3) as sp,
            tc.tile_pool(name="psp", bufs=2, space="PSUM") as psp,
        ):
            c1t = coefp.tile([128, FD], f32)
            a4t = coefp.tile([128, FD], f32)
            epst = coefp.tile([128, C], f32)
            ident = coefp.tile([128, 128], f32)
            # Coefficients go on the SWDGE queue so they stream
            # concurrently with the first X loads on the HWDGE queue.
            nc.gpsimd.dma_start(c1t[:, 0:2048], c1_in[:, 0:2048])
            nc.gpsimd.dma_start(epst[:], eps_in[:])
            nc.gpsimd.dma_start(ident[:], ident_in[:])
            nc.gpsimd.dma_start(a4t[:, 0:2048], a4_in[:, 0:2048])
            nc.gpsimd.dma_start(c1t[:, 2048:FD], c1_in[:, 2048:FD])
            nc.gpsimd.dma_start(a4t[:, 2048:FD], a4_in[:, 2048:FD])

            C1 = c1t[:]
            A4v = a4t[:].rearrange("p (c j) -> p c j", j=4)
            ONE_EPS = epst[:]

            HF = FD // 2  # 2048 columns per half
            for it in range(N_TILES):
                X = xp.tile([128, FD], f32)
                nc.sync.dma_start(X[:, 0:HF], t_tiled[it][:, 0:HF])
                nc.sync.dma_start(X[:, HF:FD], t_tiled[it][:, HF:FD])
                P = pp.tile([128, FD], f32)
                O = op.tile([128, FD], f32)
                s = sp.tile([128, C], f32)
                sb = s[:].rearrange("p (c one) -> p c one", one=1)

                for k in range(2):
                    sl = slice(k * HF, (k + 1) * HF)
                    cs = slice(k * 512, (k + 1) * 512)
                    xh = X[:, sl]
                    # V1: products for this half (+ lam*x0 in slot 0)
                    nc.vector.tensor_tensor(P[:, sl], C1[:, sl], xh, mult)
                    # V2: fold the slot-0 residual into X in place
                    xjh = xh.rearrange("p (c j) -> p j c", j=4)
                    nc.vector.tensor_tensor(
                        xjh[:, 0], xjh[:, 0], ONE_EPS[:, cs], mult
                    )
                    # V3: segmented sum of the 4 slots
                    nc.vector.tensor_reduce(
                        s[:, cs],
                        P[:, sl].rearrange("p (c j) -> p c j", j=4),
                        axis=mybir.AxisListType.X,
                        op=add,
                    )
                    # V4: T half = A4 * bcast(s~) -> PSUM
                    ps = psp.tile([128, HF], f32, tag="ps")
                    nc.vector.tensor_tensor(
                        ps[:].rearrange("p (c j) -> p c j", j=4),
                        A4v[:, cs],
                        sb[:, cs].broadcast_to([128, 512, 4]),
                        mult,
                    )
                    # PE: accumulate X' onto the half (512 cols per matmul)
                    for h in range(4):
                        nc.tensor.matmul(
                            ps[:, h * 512 : (h + 1) * 512],
                            ident[:],
                            X[:, k * HF + h * 512 : k * HF + (h + 1) * 512],
                            start=False,
                            stop=True,
                            skip_group_check=True,
                        )
                    # ACT: evict, then stream out
                    nc.scalar.copy(O[:, sl], ps[:])
                    nc.sync.dma_start(o_tiled[it][:, sl], O[:, sl])

    nc.compile()
    _CACHED["nc"] = nc
    return nc


def kernel(T: np.ndarray, Bo: np.ndarray) -> np.ndarray:
    from concourse.bass_utils import run_bass_kernel_spmd

    assert T.shape == (B_FULL, C, 4) and Bo.shape == (C, 3), (T.shape, Bo.shape)

    T = np.ascontiguousarray(T, dtype=np.float32)
    coef = _coef_from_bo(Bo)
    ident = np.eye(128, dtype=np.float32)

    nc = _build_program()

    shards = T.reshape(N_CORES, B_CORE, FD)
    in_maps = [
        {"t": shards[i], "ident": ident, **coef} for i in range(N_CORES)
    ]

    res = run_bass_kernel_spmd(nc, in_maps, core_ids=list(range(N_CORES)))

    out = np.empty((N_CORES, B_CORE, FD), dtype=np.float32)
    for i in range(N_CORES):
        out[i] = res.results[i]["o"]
    return out.reshape(B_FULL, C, 4)


# revision 21
# speedup vs baseline: 1.0522x; 1.0522x over previous
"""Trainium2 Bass kernel for the per-cluster Lorentz boost module.

out[b,c,i] = B[c,i,j] @ T[b,c,j], B derived from per-cluster boost
vectors Bo[c].  Boost matrix closed form:
    B = [[G0, -D n^T], [-D n, I + A n n^T]]
    mag = clip(|Bo|, eps, 1-eps), n = Bo/mag, g = 1/sqrt(1-mag^2)
    A = g-1, D = g*mag, G0 = 1 + A*(n.n)

Algebra (keeps every Vector-engine pass contiguous / full rate):
with lam = -D/A and s~ = lam*x0 + n.x:
    out_k = x_k + A*n_k*s~                      (k = 1..3, exact)
    out_0 = (1+eps)*x0 + (-D)*s~,  eps = (G0-1) + D*lam
Per 128-row tile:
    P    = C1 * X            (C1 = [lam, n1, n2, n3] interleaved)
    x0  *= (1+eps)           (in-place, slot-0 columns of X)
    s~   = segmented_reduce4(P)
    Tps  = A4 * bcast(s~)    (A4 = [-D, A n1, A n2, A n3]) -> PSUM chunks
    O    = Tps + X'          (PE identity-matmul accumulate onto PSUM)
    evict PSUM -> SBUF       (Scalar engine), then store DMA.
GpSimd is deliberately idle: its SW loops contend with the Vector
engine's SBUF ports (measured 2-3.6x DVE slowdown).

Sharding: pure data parallel over batch (8192 -> 8 x 1024 rows/core).
"""

import os
import sys

import numpy as np

_TRN_REPO = "/opt/trn_rl_repo"
if _TRN_REPO not in sys.path:
    sys.path.append(_TRN_REPO)

os.environ.setdefault("TRN_TYPE", "TRN2")

EPS = 1e-7

N_CORES = 8
B_FULL = 8192
B_CORE = B_FULL // N_CORES  # 1024 batch rows per core
C = 1024                    # clusters
FD = C * 4                  # free dim of a batch tile
N_TILES = B_CORE // 128     # 8 tiles of [128, 4096] per core
N_CHUNK = FD // 512         # 8 psum chunks per tile


def _coef_from_bo(Bo: np.ndarray) -> np.ndarray:
    """Per-cluster coefficients (float64 math, fp32 results), replicated
    across partitions: [C1 | A4 | 1+eps] -> (128, 2*FD + C)."""
    Bo = np.asarray(Bo, dtype=np.float32).astype(np.float64)
    mag = np.sqrt(np.sum(Bo * Bo, axis=1, keepdims=True))
    mag = np.clip(mag, EPS, 1.0 - EPS)
    n = Bo / mag                                   # (C,3)
    g = 1.0 / np.sqrt(1.0 - mag * mag)             # (C,1)
    A = g - 1.0
    D = g * mag
    nn = np.sum(n * n, axis=1, keepdims=True)
    G0 = 1.0 + A * nn
    lam32 = (-D / A).astype(np.float32)
    # eps cancels the realized -D*lam32 x0 cross-term exactly
    eps = ((G0 - 1.0) + D * lam32.astype(np.float64))

    C1 = np.empty((C, 4), dtype=np.float32)
    C1[:, 0] = lam32[:, 0]
    C1[:, 1:] = n.astype(np.float32)
    A4 = np.empty((C, 4), dtype=np.float32)
    A4[:, 0] = (-D[:, 0]).astype(np.float32)
    A4[:, 1:] = (A * n).astype(np.float32)
    one_eps = (1.0 + eps[:, 0]).astype(np.float32)

    def rep(row):
        return np.ascontiguousarray(np.broadcast_to(row.reshape(1, -1), (128, row.size)))

    return {"c1": rep(C1), "a4": rep(A4), "eps1": rep(one_eps)}


_CACHED = {}


def _build_program():
    if "nc" in _CACHED:
        return _CACHED["nc"]

    import concourse.bacc as bacc
    import concourse.mybir as mybir
    import concourse.tile as tile

    f32 = mybir.dt.float32
    mult = mybir.AluOpType.mult
    add = mybir.AluOpType.add

    nc = bacc.Bacc("TRN2", target_bir_lowering=False, debug=False)

    t_in = nc.dram_tensor("t", [B_CORE, FD], f32, kind="ExternalInput").ap()
    c1_in = nc.dram_tensor("c1", [128, FD], f32, kind="ExternalInput").ap()
    a4_in = nc.dram_tensor("a4", [128, FD], f32, kind="ExternalInput").ap()
    eps_in = nc.dram_tensor("eps1", [128, C], f32, kind="ExternalInput").ap()
    ident_in = nc.dram_tensor("ident", [128, 128], f32, kind="ExternalInput").ap()
    o_out = nc.dram_tensor("o", [B_CORE, FD], f32, kind="ExternalOutput").ap()

    t_tiled = t_in.rearrange("(n p) m -> n p m", p=128)
    o_tiled = o_out.rearrange("(n p) m -> n p m", p=128)

    with tile.TileContext(nc) as tc:
        with (
            tc.tile_pool(name="coefp", bufs=1) as coefp,
            tc.tile_pool(name="xp", bufs=3) as xp,
            tc.tile_pool(name="pp", bufs=3) as pp,
            tc.tile_pool(name="op", bufs=3) as op,
            tc.tile_pool(name="sp", bufs=3) as sp,
            tc.tile_pool(name="psp", bufs=2, space="PSUM") as psp,
        ):
            c1t = coefp.tile([128, FD], f32)
            a4t = coefp.tile([128, FD], f32)
            epst = coefp.tile([128, C], f32)
            ident = coefp.tile([128, 128], f32)
            # Ordered by when the first tile's Vector ops need them.
            nc.sync.dma_start(c1t[:, 0:2048], c1_in[:, 0:2048])
            nc.sync.dma_start(epst[:], eps_in[:])
            nc.sync.dma_start(ident[:], ident_in[:])
            nc.sync.dma_start(a4t[:, 0:2048], a4_in[:, 0:2048])
            nc.sync.dma_start(c1t[:, 2048:FD], c1_in[:, 2048:FD])
            nc.sync.dma_start(a4t[:, 2048:FD], a4_in[:, 2048:FD])

            C1 = c1t[:]
            A4v = a4t[:].rearrange("p (c j) -> p c j", j=4)
            ONE_EPS = epst[:]

            HF = FD // 2  # 2048 columns per half
            for it in range(N_TILES):
                X = xp.tile([128, FD], f32)
                nc.sync.dma_start(X[:, 0:HF], t_tiled[it][:, 0:HF])
                nc.sync.dma_start(X[:, HF:FD], t_tiled[it][:, HF:FD])
                P = pp.tile([128, FD], f32)
                O = op.tile([128, FD], f32)
                s = sp.tile([128, C], f32)
                sb = s[:].rearrange("p (c one) -> p c one", one=1)

                for k in range(2):
                    sl = slice(k * HF, (k + 1) * HF)
                    cs = slice(k * 512, (k + 1) * 512)
                    xh = X[:, sl]
                    # V1: products for this half (+ lam*x0 in slot 0)
                    nc.vector.tensor_tensor(P[:, sl], C1[:, sl], xh, mult)
                    # V2: fold the slot-0 residual into X in place
                    xjh = xh.rearrange("p (c j) -> p j c", j=4)
                    nc.vector.tensor_tensor(
                        xjh[:, 0], xjh[:, 0], ONE_EPS[:, cs], mult
                    )
                    # V3: segmented sum of the 4 slots
                    nc.vector.tensor_reduce(
                        s[:, cs],
                        P[:, sl].rearrange("p (c j) -> p c j", j=4),
                        axis=mybir.AxisListType.X,
                        op=add,
                    )
                    # V4: T half = A4 * bcast(s~) -> PSUM
                    ps = psp.tile([128, HF], f32, tag="ps")
                    nc.vector.tensor_tensor(
                        ps[:].rearrange("p (c j) -> p c j", j=4),
                        A4v[:, cs],
                        sb[:, cs].broadcast_to([128, 512, 4]),
                        mult,
                    )
                    # PE: accumulate X' onto the half (512 cols per matmul)
                    for h in range(4):
                        nc.tensor.matmul(
                            ps[:, h * 512 : (h + 1) * 512],
                            ident[:],
                            X[:, k * HF + h * 512 : k * HF + (h + 1) * 512],
                            start=False,
                            stop=True,
                            skip_group_check=True,
                        )
                    # ACT: evict, then stream out
                    nc.scalar.copy(O[:, sl], ps[:])
                    nc.sync.dma_start(o_tiled[it][:, sl], O[:, sl])

    nc.compile()
    _CACHED["nc"] = nc
    return nc


def kernel(T: np.ndarray, Bo: np.ndarray) -> np.ndarray:
    from concourse.bass_utils import run_bass_kernel_spmd

    assert T.shape == (B_FULL, C, 4) and Bo.shape == (C, 3), (T.shape, Bo.shape)

    T = np.ascontiguousarray(T, dtype=np.float32)
    coef = _coef_from_bo(Bo)
    ident = np.eye(128, dtype=np.float32)

    nc = _build_program()

    shards = T.reshape(N_CORES, B_CORE, FD)
    in_maps = [
        {"t": shards[i], "ident": ident, **coef} for i in range(N_CORES)
    ]

    res = run_bass_kernel_spmd(nc, in_maps, core_ids=list(range(N_CORES)))

    out = np.empty((N_CORES, B_CORE, FD), dtype=np.float32)
    for i in range(N_CORES):
        out[i] = res.results[i]["o"]
    return out.reshape(B_FULL, C, 4)


# revision 23
# speedup vs baseline: 1.0924x; 1.0382x over previous
"""Trainium2 Bass kernel for the per-cluster Lorentz boost module.

out[b,c,i] = B[c,i,j] @ T[b,c,j], B derived from per-cluster boost
vectors Bo[c].  Boost matrix closed form:
    B = [[G0, -D n^T], [-D n, I + A n n^T]]
    mag = clip(|Bo|, eps, 1-eps), n = Bo/mag, g = 1/sqrt(1-mag^2)
    A = g-1, D = g*mag, G0 = 1 + A*(n.n)

Algebra (keeps every Vector-engine pass contiguous / full rate):
with lam = -D/A and s~ = lam*x0 + n.x:
    out_k = x_k + A*n_k*s~                      (k = 1..3, exact)
    out_0 = (1+eps)*x0 + (-D)*s~,  eps = (G0-1) + D*lam
Per 128-row tile:
    P    = C1 * X            (C1 = [lam, n1, n2, n3] interleaved)
    x0  *= (1+eps)           (in-place, slot-0 columns of X)
    s~   = segmented_reduce4(P)
    Tps  = A4 * bcast(s~)    (A4 = [-D, A n1, A n2, A n3]) -> PSUM chunks
    O    = Tps + X'          (PE identity-matmul accumulate onto PSUM)
    evict PSUM -> SBUF       (Scalar engine), then store DMA.
GpSimd is deliberately idle: its SW loops contend with the Vector
engine's SBUF ports (measured 2-3.6x DVE slowdown).

Sharding: pure data parallel over batch (8192 -> 8 x 1024 rows/core).
"""

import os
import sys

import numpy as np

_TRN_REPO = "/opt/trn_rl_repo"
if _TRN_REPO not in sys.path:
    sys.path.append(_TRN_REPO)

os.environ.setdefault("TRN_TYPE", "TRN2")

EPS = 1e-7

N_CORES = 8
B_FULL = 8192
B_CORE = B_FULL // N_CORES  # 1024 batch rows per core
C = 1024                    # clusters
FD = C * 4                  # free dim of a batch tile
N_TILES = B_CORE // 128     # 8 tiles of [128, 4096] per core
N_CHUNK = FD // 512         # 8 psum chunks per tile


def _coef_from_bo(Bo: np.ndarray) -> np.ndarray:
    """Per-cluster coefficients (float64 math, fp32 results), replicated
    across partitions: [C1 | A4 | 1+eps] -> (128, 2*FD + C)."""
    Bo = np.asarray(Bo, dtype=np.float32).astype(np.float64)
    mag = np.sqrt(np.sum(Bo * Bo, axis=1, keepdims=True))
    mag = np.clip(mag, EPS, 1.0 - EPS)
    n = Bo / mag                                   # (C,3)
    g = 1.0 / np.sqrt(1.0 - mag * mag)             # (C,1)
    A = g - 1.0
    D = g * mag
    nn = np.sum(n * n, axis=1, keepdims=True)
    G0 = 1.0 + A * nn
    lam32 = (-D / A).astype(np.float32)
    # eps cancels the realized -D*lam32 x0 cross-term exactly
    eps = ((G0 - 1.0) + D * lam32.astype(np.float64))

    C1 = np.empty((C, 4), dtype=np.float32)
    C1[:, 0] = lam32[:, 0]
    C1[:, 1:] = n.astype(np.float32)
    A4 = np.empty((C, 4), dtype=np.float32)
    A4[:, 0] = (-D[:, 0]).astype(np.float32)
    A4[:, 1:] = (A * n).astype(np.float32)
    one_eps = (1.0 + eps[:, 0]).astype(np.float32)

    def rep(row):
        return np.ascontiguousarray(np.broadcast_to(row.reshape(1, -1), (128, row.size)))

    return {"c1": rep(C1), "a4": rep(A4), "eps1": rep(one_eps)}


_CACHED = {}


def _build_program():
    if "nc" in _CACHED:
        return _CACHED["nc"]

    import concourse.bacc as bacc
    import concourse.mybir as mybir
    import concourse.tile as tile

    f32 = mybir.dt.float32
    mult = mybir.AluOpType.mult
    add = mybir.AluOpType.add

    nc = bacc.Bacc("TRN2", target_bir_lowering=False, debug=False)

    t_in = nc.dram_tensor("t", [B_CORE, FD], f32, kind="ExternalInput").ap()
    c1_in = nc.dram_tensor("c1", [128, FD], f32, kind="ExternalInput").ap()
    a4_in = nc.dram_tensor("a4", [128, FD], f32, kind="ExternalInput").ap()
    eps_in = nc.dram_tensor("eps1", [128, C], f32, kind="ExternalInput").ap()
    ident_in = nc.dram_tensor("ident", [128, 128], f32, kind="ExternalInput").ap()
    o_out = nc.dram_tensor("o", [B_CORE, FD], f32, kind="ExternalOutput").ap()

    t_tiled = t_in.rearrange("(n p) m -> n p m", p=128)
    o_tiled = o_out.rearrange("(n p) m -> n p m", p=128)

    with tile.TileContext(nc) as tc:
        with (
            tc.tile_pool(name="coefp", bufs=1) as coefp,
            tc.tile_pool(name="xp", bufs=3) as xp,
            tc.tile_pool(name="pp", bufs=3) as pp,
            tc.tile_pool(name="op", bufs=3) as op,
            tc.tile_pool(name="sp", bufs=3) as sp,
            tc.tile_pool(name="tsbp", bufs=1) as tsbp,
            tc.tile_pool(name="psp", bufs=2, space="PSUM") as psp,
        ):
            c1t = coefp.tile([128, FD], f32)
            a4t = coefp.tile([128, FD], f32)
            epst = coefp.tile([128, C], f32)
            ident = coefp.tile([128, 128], f32)
            # Ordered by when the first tile's Vector ops need them.
            nc.sync.dma_start(c1t[:, 0:2048], c1_in[:, 0:2048])
            nc.sync.dma_start(epst[:], eps_in[:])
            nc.sync.dma_start(ident[:], ident_in[:])
            nc.sync.dma_start(a4t[:, 0:2048], a4_in[:, 0:2048])

            C1 = c1t[:]
            A4v = a4t[:].rearrange("p (c j) -> p c j", j=4)
            ONE_EPS = epst[:]

            HF = FD // 2  # 2048 columns per half
            for it in range(N_TILES):
                X = xp.tile([128, FD], f32)
                nc.sync.dma_start(X[:, 0:HF], t_tiled[it][:, 0:HF])
                nc.sync.dma_start(X[:, HF:FD], t_tiled[it][:, HF:FD])
                if it == 0:
                    # second coefficient halves ride behind tile 0's data
                    nc.sync.dma_start(c1t[:, 2048:FD], c1_in[:, 2048:FD])
                    nc.sync.dma_start(a4t[:, 2048:FD], a4_in[:, 2048:FD])
                P = pp.tile([128, FD], f32)
                O = op.tile([128, FD], f32)
                s = sp.tile([128, C], f32)
                sb = s[:].rearrange("p (c one) -> p c one", one=1)

                for k in range(2):
                    last = it == N_TILES - 1 and k == 1
                    sl = slice(k * HF, (k + 1) * HF)
                    cs = slice(k * 512, (k + 1) * 512)
                    xh = X[:, sl]
                    # V1: products for this half (+ lam*x0 in slot 0)
                    nc.vector.tensor_tensor(P[:, sl], C1[:, sl], xh, mult)
                    # V2: fold the slot-0 residual into X in place
                    xjh = xh.rearrange("p (c j) -> p j c", j=4)
                    nc.vector.tensor_tensor(
                        xjh[:, 0], xjh[:, 0], ONE_EPS[:, cs], mult
                    )
                    # V3: segmented sum of the 4 slots
                    nc.vector.tensor_reduce(
                        s[:, cs],
                        P[:, sl].rearrange("p (c j) -> p c j", j=4),
                        axis=mybir.AxisListType.X,
                        op=add,
                    )
                    if last:
                        # final half fully on V: avoids draining PE/ACT at the tail
                        tsb = tsbp.tile([128, HF], f32, tag="tsb")
                        nc.vector.tensor_tensor(
                            tsb[:].rearrange("p (c j) -> p c j", j=4),
                            A4v[:, cs],
                            sb[:, cs].broadcast_to([128, 512, 4]),
                            mult,
                        )
                        nc.vector.tensor_tensor(O[:, sl], xh, tsb[:], add)
                    else:
                        # V4: T half = A4 * bcast(s~) -> PSUM
                        ps = psp.tile([128, HF], f32, tag="ps")
                        nc.vector.tensor_tensor(
                            ps[:].rearrange("p (c j) -> p c j", j=4),
                            A4v[:, cs],
                            sb[:, cs].broadcast_to([128, 512, 4]),
                            mult,
                        )
                        # PE: accumulate X' onto the half (512 cols per matmul)
                        for h in range(4):
                            nc.tensor.matmul(
                                ps[:, h * 512 : (h + 1) * 512],
                                ident[:],
                                X[:, k * HF + h * 512 : k * HF + (h + 1) * 512],
                                start=False,
                                stop=True,
                                skip_group_check=True,
                            )
                        # ACT: evict
                        nc.scalar.copy(O[:, sl], ps[:])
                    nc.sync.dma_start(o_tiled[it][:, sl], O[:, sl])

    nc.compile()
    _CACHED["nc"] = nc
    return nc


def kernel(T: np.ndarray, Bo: np.ndarray) -> np.ndarray:
    from concourse.bass_utils import run_bass_kernel_spmd

    assert T.shape == (B_FULL, C, 4) and Bo.shape == (C, 3), (T.shape, Bo.shape)

    T = np.ascontiguousarray(T, dtype=np.float32)
    coef = _coef_from_bo(Bo)
    ident = np.eye(128, dtype=np.float32)

    nc = _build_program()

    shards = T.reshape(N_CORES, B_CORE, FD)
    in_maps = [
        {"t": shards[i], "ident": ident, **coef} for i in range(N_CORES)
    ]

    res = run_bass_kernel_spmd(nc, in_maps, core_ids=list(range(N_CORES)))

    out = np.empty((N_CORES, B_CORE, FD), dtype=np.float32)
    for i in range(N_CORES):
        out[i] = res.results[i]["o"]
    return out.reshape(B_FULL, C, 4)
